# revision 38
# baseline (speedup 1.0000x reference)
"""GATv2 (3-layer, 8-head) distributed Bass kernel for 8 Trainium2 NeuronCores.

Strategy: nodes are permuted into 392 blocks of 128 slots (round-robin by
in-degree for load balance); blocks round-robin across 8 cores. Edges (with
self-loops) are bucketed by destination block, padded to NT tiles of 128 per
block so every core runs an identical SPMD program. Per layer:
  - node phase: xl = h @ Wl (own nodes), xr = h @ Wr (own nodes)
  - xl is AllGathered to every core (all three layers)
  - edge phase per block: indirect-gather xl[src] and xr[dst], z = xl+xr,
    leaky_relu, per-head att dot -> logits, w = exp(logits) (no max-subtract:
    logits are O(1)), segment-sum via 0/1-indicator matmul on the PE array
    accumulating [num | den] in PSUM, out = num/den + b, elu (layers 0,1),
    log_softmax (layer 2).

Host side: the compiled executable (Bass build + neuronx NEFF + jax jit of
the shard_map wrapper) is memoized per process, inputs are kept
device-resident and re-uploaded only when their bytes change, so repeat
calls pay only dispatch + device exec + output download.
"""
import os
import numpy as np

os.environ.setdefault("MYCRO_LOCAL_CACHE", "1")

import jax
from jax.sharding import Mesh, NamedSharding, PartitionSpec
from jax.experimental.shard_map import shard_map

import concourse.mybir as mybir
import concourse.tile as tile
from concourse import bacc, bass2jax
from concourse.bass import IndirectOffsetOnAxis, AP

P = 128
NCORES = 8
TRACE = False
N = 50000
E = 800000
NFEAT = 128
HID = 256
H8, C32 = 8, 32
NCLASS = 47
SLOPE = 0.2

BPC = 49                      # blocks per core
NBLK = NCORES * BPC           # 392 total blocks
NCPAD = BPC * P               # 6272 padded nodes per core
NSLOT = NCORES * NCPAD        # 50176 global slots
NTO = (N + P - 1) // P        # 391 output tiles (node order)

dt = mybir.dt
f32 = dt.float32


def _layout(edge_index):
    """Host-side graph partitioning. Returns per-core edge metadata + maps."""
    src = np.concatenate([edge_index[0], np.arange(N, dtype=np.int64)])
    dst = np.concatenate([edge_index[1], np.arange(N, dtype=np.int64)])
    deg = np.bincount(dst, minlength=N)
    order = np.argsort(-deg, kind="stable")          # high-degree first
    blk_of = np.empty(N, np.int64)
    pos_of = np.empty(N, np.int64)
    idx = np.arange(N)
    blk_of[order] = idx % NBLK
    pos_of[order] = idx // NBLK
    core_of = blk_of % NCORES
    bb_of = blk_of // NCORES                          # block index within core
    gslot = core_of * NCPAD + bb_of * P + pos_of      # row in xl_full

    # bucket edges by destination block
    eb = blk_of[dst]
    cnt = np.bincount(eb, minlength=NBLK)
    NT = int(np.ceil(cnt.max() / P))
    ord_e = np.argsort(eb, kind="stable")
    src_s, dst_s = src[ord_e], dst[ord_e]
    starts = np.zeros(NBLK + 1, np.int64)
    np.cumsum(cnt, out=starts[1:])

    TPC = BPC * NT                                    # tiles per core
    src_meta = np.zeros((NCORES, TPC * P), np.int32)  # global slot of source
    dpos_meta = np.full((NCORES, TPC * P), float(P), np.float32)  # pos in block
    drow_meta = np.zeros((NCORES, TPC * P), np.int32)  # local row for xr gather
    for b in range(NBLK):
        c, bb = b % NCORES, b // NCORES
        k = cnt[b]
        sl = slice(starts[b], starts[b] + k)
        o = bb * NT * P
        src_meta[c, o:o + k] = gslot[src_s[sl]]
        dpos_meta[c, o:o + k] = pos_of[dst_s[sl]].astype(np.float32)
        drow_meta[c, o:o + k] = (bb * P + pos_of[dst_s[sl]]).astype(np.int32)
    # [128, TPC] column-major per tile: element (p, t) = edge t*128+p
    src_meta = src_meta.reshape(NCORES, TPC, P).transpose(0, 2, 1).copy()
    dpos_meta = dpos_meta.reshape(NCORES, TPC, P).transpose(0, 2, 1).copy()
    drow_meta = drow_meta.reshape(NCORES, TPC, P).transpose(0, 2, 1).copy()
    return NT, src_meta, dpos_meta, drow_meta, core_of, bb_of, pos_of, gslot


def _build(NT, sim=False, gb=False):
    """Build the SPMD Bass program (identical for all cores).

    sim=True builds a single-core collective-free variant (AllGather
    replaced by a local DRAM copy) for TimelineSim profiling only.
    """
    nc = bacc.Bacc("TRN2", target_bir_lowering=False, debug=False,
                   enable_asserts=False, num_devices=1 if sim else NCORES)
    TPC = BPC * NT

    ein = {}
    def inp(name, shape, d=f32):
        ein[name] = nc.dram_tensor(name, shape, d, kind="ExternalInput").ap()
        return ein[name]

    xTown = inp("xTown", [P, NCPAD])            # own columns of x.T (slot order)
    wl0 = inp("wl0", [NFEAT, HID]); wr0 = inp("wr0", [NFEAT, HID])
    wl1 = inp("wl1", [HID, HID]);   wr1 = inp("wr1", [HID, HID])
    wl2 = inp("wl2", [HID, NCLASS]); wr2 = inp("wr2", [HID, NCLASS])
    attb0 = inp("attb0", [P, HID]); attb1 = inp("attb1", [P, HID])
    attb2 = inp("attb2", [P, NCLASS])
    bb0 = inp("bb0", [P, HID]); bb1 = inp("bb1", [P, HID])
    bb2 = inp("bb2", [P, NCLASS])
    iota = inp("iota", [P, P])
    ident = inp("ident", [P, P])
    srcm = inp("srcm", [P, TPC], dt.int32)
    dposm = inp("dposm", [P, TPC])
    drowm = inp("drowm", [P, TPC], dt.int32)
    permm = inp("permm", [P, NTO], dt.int32)

    out_own = nc.dram_tensor("out_own", [NCPAD, NCLASS], dt.float16).ap()
    out_final = nc.dram_tensor("out_final", [NTO * P, NCLASS], dt.float16,
                               kind="ExternalOutput").ap()

    with tile.TileContext(nc) as tc:
        with tc.tile_pool(name="const", bufs=1) as cp, \
             tc.tile_pool(name="mm", bufs=3) as mp, \
             tc.tile_pool(name="gat", bufs=2) as gp, \
             tc.tile_pool(name="nps", bufs=2, space="PSUM") as nps, \
             tc.tile_pool(name="tps", bufs=2, space="PSUM") as tps, \
             tc.tile_pool(name="dram", bufs=1, space="DRAM") as dram:

            # ---- resident constants ----
            iota_sb = cp.tile([P, P], f32, tag="iota", name="iota")
            nc.sync.dma_start(iota_sb[:], iota[:])
            ident_sb = cp.tile([P, P], f32, tag="ident", name="ident")
            nc.sync.dma_start(ident_sb[:], ident[:])
            alpha_sb = cp.tile([P, 1], f32, tag="alpha", name="alpha")
            nc.gpsimd.memset(alpha_sb[:], SLOPE)
            attb_sb = [cp.tile([P, HID], dt.float16, tag="attb0", name="attb0"),
                       cp.tile([P, HID], dt.float16, tag="attb1", name="attb1"),
                       cp.tile([P, NCLASS], dt.float16, tag="attb2", name="attb2")]
            for t, s in zip(attb_sb, (attb0, attb1, attb2)):
                tf = cp.tile([P, t.shape[-1]], f32, tag="attf" + t.tensor.name,
                             name="attf")
                nc.sync.dma_start(tf[:], s[:])
                nc.vector.tensor_copy(t[:], tf[:])
            bb_sb = [cp.tile([P, HID], f32, tag="bbt0", name="bbt0"),
                     cp.tile([P, HID], f32, tag="bbt1", name="bbt1"),
                     cp.tile([P, NCLASS], f32, tag="bbt2", name="bbt2")]
            for t, s in zip(bb_sb, (bb0, bb1, bb2)):
                nc.sync.dma_start(t[:], s[:])
            w_sb = []   # weights as [K=128 subtiles][128, F] slices
            for w, kdim, fdim in ((wl0, NFEAT, HID), (wr0, NFEAT, HID),
                                  (wl1, HID, HID), (wr1, HID, HID),
                                  (wl2, HID, NCLASS), (wr2, HID, NCLASS)):
                ks = kdim // P
                t = cp.tile([P, ks, fdim], f32, tag=f"w{len(w_sb)}", name=f"w{len(w_sb)}")
                for k in range(ks):
                    nc.sync.dma_start(t[:, k, :], w[k * P:(k + 1) * P, :])
                w_sb.append(t)
            srcm_sb = cp.tile([P, TPC], dt.int32)
            nc.sync.dma_start(srcm_sb[:], srcm[:])
            dposm_sb = cp.tile([P, TPC], f32)
            nc.sync.dma_start(dposm_sb[:], dposm[:])
            drowm_sb = cp.tile([P, TPC], dt.int32)
            nc.sync.dma_start(drowm_sb[:], drowm[:])
            permm_sb = cp.tile([P, NTO], dt.int32)
            nc.sync.dma_start(permm_sb[:], permm[:])

            # ---- internal DRAM ----
            # (collective outs need Shared addr space; use raw dram tensors)
            f16 = dt.float16
            shared = {} if sim else dict(addr_space="Shared")
            xl_full = [nc.dram_tensor("xl_full0", [NSLOT, HID], f16,
                                      **shared).ap(),
                       nc.dram_tensor("xl_full1", [NSLOT, HID], f16,
                                      **shared).ap(),
                       nc.dram_tensor("xl_full2", [NSLOT, NCLASS], f16,
                                      **shared).ap()]
            xr_own = [dram.tile([NCPAD, HID], f16, tag="xr0", name="xr0"),
                      dram.tile([NCPAD, HID], f16, tag="xr1", name="xr1"),
                      dram.tile([NCPAD, NCLASS], f16, tag="xr2", name="xr2")]
            xl_bounce = [nc.dram_tensor("xl_b0", [NCPAD, HID], f16).ap(),
                         nc.dram_tensor("xl_b1", [NCPAD, HID], f16).ap(),
                         nc.dram_tensor("xl_b2", [NCPAD, NCLASS], f16).ap()]
            hT_dram = [dram.tile([HID, NCPAD], f32, tag="hT0", name="hT0"),
                       dram.tile([HID, NCPAD], f32, tag="hT1", name="hT1")]

            def node_matmuls(lhsT_feed, nk, fdim, wt, dst_dram, ntiles):
                """dst[t*128:(t+1)*128, :] = (lhsT_t).T @ W for each tile."""
                for t in range(ntiles):
                    ps = nps.tile([P, fdim], f32, space="PSUM", tag="nodeps", name="nodeps")
                    for k in range(nk):
                        nc.tensor.matmul(ps[:], lhsT_feed(t, k),
                                         wt[:, k, :],
                                         start=(k == 0), stop=(k == nk - 1))
                    o_sb = mp.tile([P, fdim], dt.float16, tag="nodeout",
                                   name="nodeout")
                    nc.vector.tensor_copy(o_sb[:], ps[:])
                    nc.sync.dma_start(dst_dram[t * P:(t + 1) * P, :], o_sb[:])

            def allgather(li):
                if sim:
                    # stand-in with comparable local DMA so TimelineSim sees
                    # a dependency + some DMA cost
                    nc.sync.dma_start(xl_full[li][:NCPAD, :], xl_bounce[li][:])
                    return
                nc.gpsimd.collective_compute(
                    "AllGather", mybir.AluOpType.bypass,
                    ins=[xl_bounce[li].opt()], outs=[xl_full[li].opt()],
                    replica_groups=[list(range(NCORES))])

            # ---- layer 0 prologue: xl0 own -> AllGather; xr0 own ----
            xTown_sb = cp.tile([P, NCPAD], f32)
            nc.sync.dma_start(xTown_sb[:], xTown[:])
            node_matmuls(lambda t, k: xTown_sb[:, t * P:(t + 1) * P], 1, HID,
                         w_sb[0], xl_bounce[0], BPC)
            allgather(0)
            node_matmuls(lambda t, k: xTown_sb[:, t * P:(t + 1) * P], 1, HID,
                         w_sb[1], xr_own[0], BPC)

            # ---- per-layer edge phase ----
            def edge_phase(li, F, nh, chan):
                """Process all blocks for layer li. F=feat width, heads nh*chan=F."""
                FD = F + nh  # rhs width: scaled | w
                # layers 0/1: 2 groups (SBUF budget); layer 2 (F=47): 1 group
                NTH = (NT + 1) // 2
                CH = 6 if li == 2 else 5  # gather tiles per SWDGE instruction
                tg = str(li == 2)
                for bb in range(BPC):
                    num_ps = nps.tile([P, FD], f32, space="PSUM", tag="numps", name="numps")
                    for g0 in range(0, NT, NTH):
                        nth = min(NTH, NT - g0)
                        xl_g = gp.tile([P, NTH, F], dt.float16, tag="xlg" + tg,
                                       name="xlg")
                        xr_g = gp.tile([P, NTH, F], dt.float16, tag="xrg" + tg,
                                       name="xrg")
                        # batched indirect gathers: SWDGE cost is ~994ns fixed
                        # per instruction + 0.34ns/descriptor, so pack up to 5
                        # tiles (640 descriptors) per instruction
                        for j0 in range(0, nth, CH if gb else 1):
                            bt = min(CH, nth - j0) if gb else 1
                            tc0 = bb * NT + g0 + j0
                            xl_o = xl_g[:, j0:j0 + bt, :] if gb else xl_g[:, j0, :]
                            xr_o = xr_g[:, j0:j0 + bt, :] if gb else xr_g[:, j0, :]
                            nc.gpsimd.indirect_dma_start(
                                out=xl_o, out_offset=None,
                                in_=xl_full[li][:],
                                in_offset=IndirectOffsetOnAxis(
                                    ap=srcm_sb[:, tc0:tc0 + bt], axis=0))
                            nc.gpsimd.indirect_dma_start(
                                out=xr_o, out_offset=None,
                                in_=xr_own[li][:],
                                in_offset=IndirectOffsetOnAxis(
                                    ap=drowm_sb[:, tc0:tc0 + bt], axis=0))
                        # indicator IT[p, jj, n] = (iota[n] == dpos[p, col])
                        it_sb = gp.tile([P, NTH, P], dt.float16, tag="it" + tg,
                                        name="it")
                        iota_b = AP(iota_sb.tensor, iota_sb.offset,
                                    [iota_sb.ap[0], [0, nth], [1, P]])
                        dp = dposm_sb[:, bb * NT + g0:bb * NT + g0 + nth]
                        dpos_b = AP(dp.tensor, dp.offset, [dp.ap[0], [1, nth], [0, P]])
                        nc.vector.tensor_tensor(out=it_sb[:, :nth, :], in0=iota_b,
                                                in1=dpos_b,
                                                op=mybir.AluOpType.is_equal)
                        # z = xl + xr into zl_sb (no aliasing on DVE)
                        zl_sb = gp.tile([P, NTH, F], dt.float16, tag="zl" + tg,
                                        name="zl")
                        nc.vector.tensor_tensor(out=zl_sb[:, :nth, :],
                                                in0=xl_g[:, :nth, :],
                                                in1=xr_g[:, :nth, :],
                                                op=mybir.AluOpType.add)
                        # leaky relu via Prelu with alpha AP; xr_g is dead,
                        # reuse it for the Prelu output
                        nc.scalar.activation(xr_g[:, :nth, :], zl_sb[:, :nth, :],
                                             mybir.ActivationFunctionType.Prelu,
                                             alpha=alpha_sb[:])
                        # zw = zl * att (into xr_g scratch), logits = sum_c zw
                        ab = attb_sb[li]
                        attb_4d = AP(ab.tensor, ab.offset,
                                     [ab.ap[0], [0, nth], [chan, nh], [1, chan]])
                        zl_4d = AP(xr_g.tensor, xr_g.offset,
                                   [xr_g.ap[0], [F, nth], [chan, nh], [1, chan]])
                        zw_4d = AP(zl_sb.tensor, zl_sb.offset,
                                   [zl_sb.ap[0], [F, nth], [chan, nh], [1, chan]])
                        nc.vector.tensor_tensor(out=zw_4d, in0=zl_4d, in1=attb_4d,
                                                op=mybir.AluOpType.mult)
                        logit_sb = gp.tile([P, NTH, nh], f32, tag="logit" + tg, name="logit")
                        nc.vector.tensor_reduce(logit_sb[:, :nth, :], zw_4d,
                                                axis=mybir.AxisListType.X,
                                                op=mybir.AluOpType.add)
                        # rhs = [xl*w | w]
                        rhs_sb = gp.tile([P, NTH, FD], dt.float16, tag="rhs" + tg,
                                         name="rhs")
                        nc.scalar.activation(rhs_sb[:, :nth, F:FD],
                                             logit_sb[:, :nth, :],
                                             mybir.ActivationFunctionType.Exp)
                        w_b = AP(rhs_sb.tensor, rhs_sb.offset + F,
                                 [rhs_sb.ap[0], [FD, nth], [1, nh], [0, chan]])
                        xl_4d = AP(xl_g.tensor, xl_g.offset,
                                   [xl_g.ap[0], [F, nth], [chan, nh], [1, chan]])
                        rhs_4d = AP(rhs_sb.tensor, rhs_sb.offset,
                                    [rhs_sb.ap[0], [FD, nth], [chan, nh], [1, chan]])
                        nc.vector.tensor_tensor(out=rhs_4d, in0=xl_4d, in1=w_b,
                                                op=mybir.AluOpType.mult)
                        # segment matmul: [num | den] accumulated over NT tiles
                        for jj in range(nth):
                            j = g0 + jj
                            nc.tensor.matmul(num_ps[:],
                                             it_sb[:, jj, :],
                                             rhs_sb[:, jj, :],
                                             start=(j == 0), stop=(j == NT - 1))
                    # out = num / max(den, tiny) + bias
                    den_sb = gp.tile([P, nh], f32, tag="den", name="den")
                    nc.vector.tensor_scalar_max(den_sb[:], num_ps[:, F:FD], 1e-30)
                    rec_sb = gp.tile([P, nh], f32, tag="rec", name="rec")
                    nc.vector.reciprocal(rec_sb[:], den_sb[:])
                    ov_sb = gp.tile([P, F], f32, tag="ov", name="ov")
                    rec_b = AP(rec_sb.tensor, rec_sb.offset,
                               [rec_sb.ap[0], [1, nh], [0, chan]])
                    num_3d = AP(num_ps.tensor, num_ps.offset,
                                [num_ps.ap[0], [chan, nh], [1, chan]])
                    nc.vector.tensor_tensor(
                        out=AP(ov_sb.tensor, ov_sb.offset,
                               [ov_sb.ap[0], [chan, nh], [1, chan]]),
                        in0=num_3d, in1=rec_b, op=mybir.AluOpType.mult)
                    hv_sb = gp.tile([P, F], f32, tag="hv", name="hv")
                    nc.vector.tensor_tensor(out=hv_sb[:], in0=ov_sb[:],
                                            in1=bb_sb[li][:],
                                            op=mybir.AluOpType.add)
                    if li < 2:
                        # elu = relu(h) + exp(min(h,0)) - 1, then h^T to DRAM
                        mn_sb = gp.tile([P, F], f32, tag="mn", name="mn")
                        nc.vector.tensor_scalar_min(mn_sb[:], hv_sb[:], 0.0)
                        ex_sb = gp.tile([P, F], f32, tag="ex", name="ex")
                        nc.scalar.activation(ex_sb[:], mn_sb[:],
                                             mybir.ActivationFunctionType.Exp)
                        rl_sb = gp.tile([P, F], f32, tag="rl", name="rl")
                        nc.scalar.activation(rl_sb[:], hv_sb[:],
                                             mybir.ActivationFunctionType.Relu)
                        el_sb = gp.tile([P, F], f32, tag="el", name="el")
                        nc.vector.tensor_tensor(out=el_sb[:], in0=rl_sb[:],
                                                in1=ex_sb[:],
                                                op=mybir.AluOpType.add)
                        nc.vector.tensor_scalar_add(el_sb[:], el_sb[:], -1.0)
                        for half in range(2):
                            tp_ps = tps.tile([P, P], f32, space="PSUM", tag="tp", name="tp")
                            nc.tensor.transpose(
                                tp_ps[:], el_sb[:, half * P:(half + 1) * P],
                                ident_sb[:])
                            tp_sb = gp.tile([P, P], f32, tag="tpsb", name="tpsb")
                            nc.vector.tensor_copy(tp_sb[:], tp_ps[:])
                            nc.sync.dma_start(
                                hT_dram[li][half * P:(half + 1) * P,
                                            bb * P:(bb + 1) * P], tp_sb[:])
                    else:
                        # log_softmax over 47 classes
                        mx_sb = gp.tile([P, 1], f32, tag="mx", name="mx")
                        nc.vector.tensor_reduce(mx_sb[:], hv_sb[:],
                                                axis=mybir.AxisListType.X,
                                                op=mybir.AluOpType.max,
                                                negate=True)
                        e2_sb = gp.tile([P, F], f32, tag="e2", name="e2")
                        sm_sb = gp.tile([P, 1], f32, tag="sm", name="sm")
                        nc.scalar.activation(e2_sb[:, :NCLASS], hv_sb[:],
                                             mybir.ActivationFunctionType.Exp,
                                             bias=mx_sb[:], accum_out=sm_sb[:])
                        ln_sb = gp.tile([P, 1], f32, tag="ln", name="ln")
                        nc.scalar.activation(ln_sb[:], sm_sb[:],
                                             mybir.ActivationFunctionType.Ln)
                        sh_sb = gp.tile([P, 1], f32, tag="sh", name="sh")
                        nc.vector.tensor_tensor(out=sh_sb[:], in0=mx_sb[:],
                                                in1=ln_sb[:],
                                                op=mybir.AluOpType.subtract)
                        fo_sb = gp.tile([P, F], dt.float16, tag="fo", name="fo")
                        nc.vector.tensor_scalar(fo_sb[:, :NCLASS], hv_sb[:],
                                                sh_sb[:], None,
                                                op0=mybir.AluOpType.add)
                        nc.sync.dma_start(out_own[bb * P:(bb + 1) * P, :],
                                          fo_sb[:, :NCLASS])

            edge_phase(0, HID, H8, C32)

            # ---- node phase layer 1 + AllGather ----
            def feed_hT(li):
                def f(t, k):
                    s = mp.tile([P, P], f32, tag="hfeed", name="hfeed")
                    nc.sync.dma_start(
                        s[:], hT_dram[li][k * P:(k + 1) * P, t * P:(t + 1) * P])
                    return s[:]
                return f
            node_matmuls(feed_hT(0), 2, HID, w_sb[2], xl_bounce[1], BPC)
            allgather(1)
            node_matmuls(feed_hT(0), 2, HID, w_sb[3], xr_own[1], BPC)

            edge_phase(1, HID, H8, C32)

            node_matmuls(feed_hT(1), 2, NCLASS, w_sb[4], xl_bounce[2], BPC)
            allgather(2)
            node_matmuls(feed_hT(1), 2, NCLASS, w_sb[5], xr_own[2], BPC)

            edge_phase(2, NCLASS, 1, NCLASS)

            # ---- gather all per-core outputs + permute to node order ----
            # (host then fetches a single core's out_final shard: 1 RPC)
            out_slots = nc.dram_tensor("out_slots", [NSLOT, NCLASS],
                                       dt.float16, **shared).ap()
            if sim:
                nc.sync.dma_start(out_slots[:NCPAD, :], out_own[:])
            else:
                nc.gpsimd.collective_compute(
                    "AllGather", mybir.AluOpType.bypass,
                    ins=[out_own.opt()], outs=[out_slots.opt()],
                    replica_groups=[list(range(NCORES))])
            if gb:
                OCH = 6
                for t0 in range(0, NTO, OCH):
                    bt = min(OCH, NTO - t0)
                    og = gp.tile([P, OCH, NCLASS], dt.float16, tag="og", name="og")
                    nc.gpsimd.indirect_dma_start(
                        out=og[:, :bt, :], out_offset=None, in_=out_slots[:],
                        in_offset=IndirectOffsetOnAxis(
                            ap=permm_sb[:, t0:t0 + bt], axis=0))
                    ov = AP(out_final.tensor,
                            out_final.offset + t0 * P * NCLASS,
                            [[NCLASS, P], [P * NCLASS, bt], [1, NCLASS]])
                    nc.sync.dma_start(ov, og[:, :bt, :])
            else:
                for t in range(NTO):
                    og = gp.tile([P, NCLASS], dt.float16, tag="og", name="og")
                    nc.gpsimd.indirect_dma_start(
                        out=og[:], out_offset=None, in_=out_slots[:],
                        in_offset=IndirectOffsetOnAxis(
                            ap=permm_sb[:, t:t + 1], axis=0))
                    nc.sync.dma_start(out_final[t * P:(t + 1) * P, :], og[:])

    nc.compile()
    return nc


# ---------------------------------------------------------------------------
# Cached executor: build + jit once per NT; inputs stay device-resident.
# ---------------------------------------------------------------------------

_EXEC_CACHE = {}
_STATE = {"edge_copy": None, "layout": None, "perm": None,
          "in_copy": None, "dev_args": None, "dev_NT": None}


def _same(a, b):
    return a is not None and a.shape == b.shape and a.dtype == b.dtype \
        and np.array_equal(a, b)


def _get_exec(NT):
    if NT in _EXEC_CACHE:
        return _EXEC_CACHE[NT]
    bass2jax.install_neuronx_cc_hook()
    nc = _build(NT)

    partition_name = (nc.partition_id_tensor.name
                      if nc.partition_id_tensor else None)
    in_names, out_names, out_avals = [], [], []
    for alloc in nc.m.functions[0].allocations:
        if not isinstance(alloc, mybir.MemoryLocationSet):
            continue
        name = alloc.memorylocations[0].name
        if alloc.kind == "ExternalInput":
            if name != partition_name:
                in_names.append(name)
        elif alloc.kind == "ExternalOutput":
            out_names.append(name)
            out_avals.append(jax.core.ShapedArray(
                tuple(alloc.tensor_shape), mybir.dt.np(alloc.dtype)))
    assert nc.dbg_addr is None
    n_params, n_outs = len(in_names), len(out_names)
    all_in = tuple(in_names + out_names +
                   ([partition_name] if partition_name else []))

    def _body(*args):
        operands = list(args)
        if partition_name is not None:
            operands.append(bass2jax.partition_id_tensor())
        outs = bass2jax._bass_exec_p.bind(
            *operands, out_avals=tuple(out_avals), in_names=all_in,
            out_names=tuple(out_names), lowering_input_output_aliases=(),
            sim_require_finite=True, sim_require_nnan=True, nc=nc)
        return tuple(outs)

    devices = jax.devices()[:NCORES]
    mesh = Mesh(np.asarray(devices), ("core",))
    spec = PartitionSpec("core")
    sharding = NamedSharding(mesh, spec)
    sharded = jax.jit(
        shard_map(_body, mesh=mesh, in_specs=(spec,) * (n_params + n_outs),
                  out_specs=(spec,) * n_outs, check_rep=False),
        keep_unused=True)

    # The NEFF writes every element of out_own, so the "output" operands
    # need not be donated — keep one persistent zeros array device-resident
    # and pass it on every call.
    zinfo = [(tuple(a.shape), a.dtype) for a in out_avals]
    zeros_arr = tuple(
        jax.device_put(np.zeros((NCORES * s[0],) + s[1:], d), sharding)
        for s, d in zinfo)

    ex = dict(nc=nc, jit=sharded, zeros_arr=zeros_arr, in_names=in_names,
              out_names=out_names, out_avals=out_avals, sharding=sharding)
    _EXEC_CACHE[NT] = ex
    return ex


def _concat_inputs(ex, user, layout):
    """Build the global (8*rows, ...) arrays in in_names order."""
    NT, src_m, dpos_m, drow_m, core_of, bb_of, pos_of, gslot = layout
    TPC = BPC * NT
    xT = np.zeros((P, NSLOT), np.float32)
    xT[:, gslot] = user["x"].T
    xTown_g = np.ascontiguousarray(
        xT.reshape(P, NCORES, NCPAD).transpose(1, 0, 2)).reshape(
            NCORES * P, NCPAD)

    def rep(a):
        a = np.ascontiguousarray(np.asarray(a, np.float32))
        return np.ascontiguousarray(
            np.broadcast_to(a[None], (NCORES,) + a.shape)).reshape(
                (NCORES * a.shape[0],) + a.shape[1:])

    def bc(a, w):
        return np.broadcast_to(
            np.asarray(a, np.float32).reshape(1, w), (P, w))

    iota = np.broadcast_to(np.arange(P, dtype=np.float32)[None, :], (P, P))
    arrs = {
        "xTown": xTown_g,
        "wl0": rep(user["Wl0"]), "wr0": rep(user["Wr0"]),
        "wl1": rep(user["Wl1"]), "wr1": rep(user["Wr1"]),
        "wl2": rep(user["Wl2"]), "wr2": rep(user["Wr2"]),
        "attb0": rep(bc(user["a0"], HID)), "attb1": rep(bc(user["a1"], HID)),
        "attb2": rep(bc(user["a2"], NCLASS)),
        "bb0": rep(bc(user["b0"], HID)), "bb1": rep(bc(user["b1"], HID)),
        "bb2": rep(bc(user["b2"], NCLASS)),
        "iota": rep(iota), "ident": rep(np.eye(P, dtype=np.float32)),
        "srcm": src_m.reshape(NCORES * P, TPC),
        "dposm": dpos_m.reshape(NCORES * P, TPC),
        "drowm": drow_m.reshape(NCORES * P, TPC),
    }
    pflat = np.zeros(NTO * P, np.int32)
    pflat[:N] = (core_of * NCPAD + bb_of * P + pos_of).astype(np.int32)
    pm = np.ascontiguousarray(pflat.reshape(NTO, P).T)        # [P, NTO]
    arrs["permm"] = np.ascontiguousarray(
        np.broadcast_to(pm[None], (NCORES,) + pm.shape)).reshape(
            NCORES * P, NTO)
    host = [arrs[n] for n in ex["in_names"]]
    return jax.device_put(host, [ex["sharding"]] * len(host))


def kernel(x, edge_index, Wl0, Wr0, a0, b0, Wl1, Wr1, a1, b1, Wl2, Wr2, a2, b2):
    x = np.ascontiguousarray(np.asarray(x, np.float32))
    edge_index = np.ascontiguousarray(np.asarray(edge_index))
    user = dict(x=x, Wl0=Wl0, Wr0=Wr0, a0=a0, b0=b0, Wl1=Wl1, Wr1=Wr1,
                a1=a1, b1=b1, Wl2=Wl2, Wr2=Wr2, a2=a2, b2=b2)
    user = {k: np.ascontiguousarray(np.asarray(v, np.float32))
            for k, v in user.items()}
    st = _STATE

    # Optimistic dispatch: if we have cached device inputs from a previous
    # call, launch the NEFF immediately and validate the cache against the
    # new inputs while the device runs. On mismatch the speculative result
    # is discarded and we re-dispatch with fresh uploads.
    outs = None
    if st["dev_args"] is not None:
        ex = _get_exec(st["dev_NT"])
        outs = ex["jit"](*st["dev_args"], *ex["zeros_arr"])

    edge_ok = _same(st["edge_copy"], edge_index)
    if not edge_ok:
        st["layout"] = _layout(edge_index)
        st["edge_copy"] = edge_index.copy()
        st["dev_args"] = None

    NT = st["layout"][0]
    ex = _get_exec(NT)

    if (st["dev_args"] is None or st["dev_NT"] != NT or
            st["in_copy"] is None or
            not all(_same(st["in_copy"].get(k), user[k]) for k in user)):
        st["dev_args"] = _concat_inputs(ex, user, st["layout"])
        st["in_copy"] = {k: v.copy() for k, v in user.items()}
        st["dev_NT"] = NT
        outs = None  # speculative result was stale

    # out_final is replicated across cores (post-AllGather + on-device
    # permute); fetch a single core's shard — one transfer RPC. Retry once
    # on transient device/transport errors.
    last_err = None
    for attempt in range(2):
        try:
            if outs is None:
                outs = ex["jit"](*st["dev_args"], *ex["zeros_arr"])
            res = np.asarray(outs[0].addressable_shards[0].data)
            return res[:N].astype(np.float32)
        except Exception as e:                       # noqa: BLE001
            last_err = e
            outs = None
            import time
            time.sleep(2.0)
    raise last_err


# revision 40
# speedup vs baseline: 1.1757x; 1.1757x over previous
"""GATv2 (3-layer, 8-head) distributed Bass kernel for 8 Trainium2 NeuronCores.

Strategy: nodes are permuted into 392 blocks of 128 slots (round-robin by
in-degree for load balance); blocks round-robin across 8 cores. Edges (with
self-loops) are bucketed by destination block, padded to NT tiles of 128 per
block so every core runs an identical SPMD program. Per layer:
  - node phase: xl = h @ Wl (own nodes), xr = h @ Wr (own nodes)
  - xl is AllGathered to every core (all three layers)
  - edge phase per block: indirect-gather xl[src] and xr[dst], z = xl+xr,
    leaky_relu, per-head att dot -> logits, w = exp(logits) (no max-subtract:
    logits are O(1)), segment-sum via 0/1-indicator matmul on the PE array
    accumulating [num | den] in PSUM, out = num/den + b, elu (layers 0,1),
    log_softmax (layer 2).

Host side: the compiled executable (Bass build + neuronx NEFF + jax jit of
the shard_map wrapper) is memoized per process, inputs are kept
device-resident and re-uploaded only when their bytes change, so repeat
calls pay only dispatch + device exec + output download.
"""
import os
import numpy as np

os.environ.setdefault("MYCRO_LOCAL_CACHE", "1")

import jax
from jax.sharding import Mesh, NamedSharding, PartitionSpec
from jax.experimental.shard_map import shard_map

import concourse.mybir as mybir
import concourse.tile as tile
from concourse import bacc, bass2jax
from concourse.bass import IndirectOffsetOnAxis, AP

P = 128
NCORES = 8
TRACE = False
N = 50000
E = 800000
NFEAT = 128
HID = 256
H8, C32 = 8, 32
NCLASS = 47
SLOPE = 0.2

BPC = 49                      # blocks per core
NBLK = NCORES * BPC           # 392 total blocks
NCPAD = BPC * P               # 6272 padded nodes per core
NSLOT = NCORES * NCPAD        # 50176 global slots
NTO = (N + P - 1) // P        # 391 output tiles (node order)

dt = mybir.dt
f32 = dt.float32


def _layout(edge_index):
    """Host-side graph partitioning. Returns per-core edge metadata + maps."""
    src = np.concatenate([edge_index[0], np.arange(N, dtype=np.int64)])
    dst = np.concatenate([edge_index[1], np.arange(N, dtype=np.int64)])
    deg = np.bincount(dst, minlength=N)
    order = np.argsort(-deg, kind="stable")          # high-degree first
    blk_of = np.empty(N, np.int64)
    pos_of = np.empty(N, np.int64)
    idx = np.arange(N)
    blk_of[order] = idx % NBLK
    pos_of[order] = idx // NBLK
    core_of = blk_of % NCORES
    bb_of = blk_of // NCORES                          # block index within core
    gslot = core_of * NCPAD + bb_of * P + pos_of      # row in xl_full

    # bucket edges by destination block
    eb = blk_of[dst]
    cnt = np.bincount(eb, minlength=NBLK)
    NT = int(np.ceil(cnt.max() / P))
    ord_e = np.argsort(eb, kind="stable")
    src_s, dst_s = src[ord_e], dst[ord_e]
    starts = np.zeros(NBLK + 1, np.int64)
    np.cumsum(cnt, out=starts[1:])

    TPC = BPC * NT                                    # tiles per core
    src_meta = np.zeros((NCORES, TPC * P), np.int32)  # global slot of source
    dpos_meta = np.full((NCORES, TPC * P), float(P), np.float32)  # pos in block
    drow_meta = np.zeros((NCORES, TPC * P), np.int32)  # local row for xr gather
    for b in range(NBLK):
        c, bb = b % NCORES, b // NCORES
        k = cnt[b]
        sl = slice(starts[b], starts[b] + k)
        o = bb * NT * P
        src_meta[c, o:o + k] = gslot[src_s[sl]]
        dpos_meta[c, o:o + k] = pos_of[dst_s[sl]].astype(np.float32)
        drow_meta[c, o:o + k] = (bb * P + pos_of[dst_s[sl]]).astype(np.int32)
    # [128, TPC] column-major per tile: element (p, t) = edge t*128+p
    src_meta = src_meta.reshape(NCORES, TPC, P).transpose(0, 2, 1).copy()
    dpos_meta = dpos_meta.reshape(NCORES, TPC, P).transpose(0, 2, 1).copy()
    drow_meta = drow_meta.reshape(NCORES, TPC, P).transpose(0, 2, 1).copy()
    return NT, src_meta, dpos_meta, drow_meta, core_of, bb_of, pos_of, gslot


def _build(NT, sim=False, gb=False):
    """Build the SPMD Bass program (identical for all cores).

    sim=True builds a single-core collective-free variant (AllGather
    replaced by a local DRAM copy) for TimelineSim profiling only.
    """
    nc = bacc.Bacc("TRN2", target_bir_lowering=False, debug=False,
                   enable_asserts=False, num_devices=1 if sim else NCORES)
    TPC = BPC * NT

    ein = {}
    def inp(name, shape, d=f32):
        ein[name] = nc.dram_tensor(name, shape, d, kind="ExternalInput").ap()
        return ein[name]

    xTown = inp("xTown", [P, NCPAD])            # own columns of x.T (slot order)
    wl0 = inp("wl0", [NFEAT, HID]); wr0 = inp("wr0", [NFEAT, HID])
    wl1 = inp("wl1", [HID, HID]);   wr1 = inp("wr1", [HID, HID])
    wl2 = inp("wl2", [HID, NCLASS]); wr2 = inp("wr2", [HID, NCLASS])
    attb0 = inp("attb0", [P, HID]); attb1 = inp("attb1", [P, HID])
    attb2 = inp("attb2", [P, NCLASS])
    bb0 = inp("bb0", [P, HID]); bb1 = inp("bb1", [P, HID])
    bb2 = inp("bb2", [P, NCLASS])
    iota = inp("iota", [P, P])
    ident = inp("ident", [P, P])
    srcm = inp("srcm", [P, TPC], dt.int32)
    dposm = inp("dposm", [P, TPC])
    drowm = inp("drowm", [P, TPC], dt.int32)
    permm = inp("permm", [P, NTO], dt.int32)

    out_own = nc.dram_tensor("out_own", [NCPAD, NCLASS], dt.float16).ap()
    out_final = nc.dram_tensor("out_final", [NTO * P, NCLASS], dt.float16,
                               kind="ExternalOutput").ap()

    with tile.TileContext(nc) as tc:
        with tc.tile_pool(name="const", bufs=1) as cp, \
             tc.tile_pool(name="mm", bufs=3) as mp, \
             tc.tile_pool(name="gat", bufs=3) as gp, \
             tc.tile_pool(name="nps", bufs=3, space="PSUM") as nps, \
             tc.tile_pool(name="tps", bufs=2, space="PSUM") as tps, \
             tc.tile_pool(name="dram", bufs=1, space="DRAM") as dram:

            # ---- resident constants ----
            iota_sb = cp.tile([P, P], f32, tag="iota", name="iota")
            nc.sync.dma_start(iota_sb[:], iota[:])
            ident_sb = cp.tile([P, P], f32, tag="ident", name="ident")
            nc.sync.dma_start(ident_sb[:], ident[:])
            alpha_sb = cp.tile([P, 1], f32, tag="alpha", name="alpha")
            nc.gpsimd.memset(alpha_sb[:], SLOPE)
            attb_sb = [cp.tile([P, HID], dt.float16, tag="attb0", name="attb0"),
                       cp.tile([P, HID], dt.float16, tag="attb1", name="attb1"),
                       cp.tile([P, NCLASS], dt.float16, tag="attb2", name="attb2")]
            for t, s in zip(attb_sb, (attb0, attb1, attb2)):
                tf = cp.tile([P, t.shape[-1]], f32, tag="attf" + t.tensor.name,
                             name="attf")
                nc.sync.dma_start(tf[:], s[:])
                nc.vector.tensor_copy(t[:], tf[:])
            bb_sb = [cp.tile([P, HID], f32, tag="bbt0", name="bbt0"),
                     cp.tile([P, HID], f32, tag="bbt1", name="bbt1"),
                     cp.tile([P, NCLASS], f32, tag="bbt2", name="bbt2")]
            for t, s in zip(bb_sb, (bb0, bb1, bb2)):
                nc.sync.dma_start(t[:], s[:])
            w_sb = []   # weights as [K=128 subtiles][128, F] f16 slices
            for w, kdim, fdim in ((wl0, NFEAT, HID), (wr0, NFEAT, HID),
                                  (wl1, HID, HID), (wr1, HID, HID),
                                  (wl2, HID, NCLASS), (wr2, HID, NCLASS)):
                ks = kdim // P
                t = cp.tile([P, ks, fdim], dt.float16, tag=f"w{len(w_sb)}",
                            name=f"w{len(w_sb)}")
                for k in range(ks):
                    wf = mp.tile([P, fdim], f32, tag="wload", name="wload")
                    nc.sync.dma_start(wf[:], w[k * P:(k + 1) * P, :])
                    nc.vector.tensor_copy(t[:, k, :], wf[:])
                w_sb.append(t)
            srcm_sb = cp.tile([P, TPC], dt.int32)
            nc.sync.dma_start(srcm_sb[:], srcm[:])
            dposm_sb = cp.tile([P, TPC], f32)
            nc.sync.dma_start(dposm_sb[:], dposm[:])
            drowm_sb = cp.tile([P, TPC], dt.int32)
            nc.sync.dma_start(drowm_sb[:], drowm[:])
            permm_sb = cp.tile([P, NTO], dt.int32)
            nc.sync.dma_start(permm_sb[:], permm[:])

            # ---- internal DRAM ----
            # (collective outs need Shared addr space; use raw dram tensors)
            f16 = dt.float16
            shared = {} if sim else dict(addr_space="Shared")
            xl_full = [nc.dram_tensor("xl_full0", [NSLOT, HID], f16,
                                      **shared).ap(),
                       nc.dram_tensor("xl_full1", [NSLOT, HID], f16,
                                      **shared).ap(),
                       nc.dram_tensor("xl_full2", [NSLOT, NCLASS], f16,
                                      **shared).ap()]
            xr_own = [dram.tile([NCPAD, HID], f16, tag="xr0", name="xr0"),
                      dram.tile([NCPAD, HID], f16, tag="xr1", name="xr1"),
                      dram.tile([NCPAD, NCLASS], f16, tag="xr2", name="xr2")]
            xl_bounce = [nc.dram_tensor("xl_b0", [NCPAD, HID], f16).ap(),
                         nc.dram_tensor("xl_b1", [NCPAD, HID], f16).ap(),
                         nc.dram_tensor("xl_b2", [NCPAD, NCLASS], f16).ap()]
            hT_dram = [dram.tile([HID, NCPAD], dt.float16, tag="hT0", name="hT0"),
                       dram.tile([HID, NCPAD], dt.float16, tag="hT1", name="hT1")]

            def node_matmuls(lhsT_feed, nk, fdim, wt, dst_dram, ntiles):
                """dst[t*128:(t+1)*128, :] = (lhsT_t).T @ W for each tile."""
                for t in range(ntiles):
                    ps = nps.tile([P, fdim], f32, space="PSUM", tag="nodeps", name="nodeps")
                    for k in range(nk):
                        nc.tensor.matmul(ps[:], lhsT_feed(t, k),
                                         wt[:, k, :],
                                         start=(k == 0), stop=(k == nk - 1))
                    o_sb = mp.tile([P, fdim], dt.float16, tag="nodeout",
                                   name="nodeout")
                    nc.vector.tensor_copy(o_sb[:], ps[:])
                    nc.sync.dma_start(dst_dram[t * P:(t + 1) * P, :], o_sb[:])

            def allgather(li):
                if sim:
                    # stand-in with comparable local DMA so TimelineSim sees
                    # a dependency + some DMA cost
                    nc.sync.dma_start(xl_full[li][:NCPAD, :], xl_bounce[li][:])
                    return
                nc.gpsimd.collective_compute(
                    "AllGather", mybir.AluOpType.bypass,
                    ins=[xl_bounce[li].opt()], outs=[xl_full[li].opt()],
                    replica_groups=[list(range(NCORES))])

            # ---- layer 0 prologue: xl0 own -> AllGather; xr0 own ----
            xTown_sb = cp.tile([P, NCPAD], dt.float16, tag="xT16", name="xT16")
            for t in range(BPC):
                xf = mp.tile([P, P], f32, tag="xload", name="xload")
                nc.sync.dma_start(xf[:], xTown[:, t * P:(t + 1) * P])
                nc.vector.tensor_copy(xTown_sb[:, t * P:(t + 1) * P], xf[:])
            node_matmuls(lambda t, k: xTown_sb[:, t * P:(t + 1) * P], 1, HID,
                         w_sb[0], xl_bounce[0], BPC)
            allgather(0)
            node_matmuls(lambda t, k: xTown_sb[:, t * P:(t + 1) * P], 1, HID,
                         w_sb[1], xr_own[0], BPC)

            # ---- per-layer edge phase ----
            def edge_phase(li, F, nh, chan):
                """Process all blocks for layer li. F=feat width, heads nh*chan=F."""
                FD = F + nh  # rhs width: scaled | w
                # layers 0/1: 2 groups (SBUF budget); layer 2 (F=47): 1 group
                NTH = (NT + 1) // 2
                CH = 6 if li == 2 else 5  # gather tiles per SWDGE instruction
                tg = str(li == 2)
                for bb in range(BPC):
                    num_ps = nps.tile([P, FD], f32, space="PSUM", tag="numps", name="numps")
                    for g0 in range(0, NT, NTH):
                        nth = min(NTH, NT - g0)
                        xl_g = gp.tile([P, NTH, F], dt.float16, tag="xlg" + tg,
                                       name="xlg")
                        xr_g = gp.tile([P, NTH, F], dt.float16, tag="xrg" + tg,
                                       name="xrg")
                        # batched indirect gathers: SWDGE cost is ~994ns fixed
                        # per instruction + 0.34ns/descriptor, so pack up to 5
                        # tiles (640 descriptors) per instruction
                        for j0 in range(0, nth, CH if gb else 1):
                            bt = min(CH, nth - j0) if gb else 1
                            tc0 = bb * NT + g0 + j0
                            if gb == "flat":
                                # batched gather, flat contiguous 2D dest
                                xl_o = AP(xl_g.tensor, xl_g.offset + j0 * F,
                                          [xl_g.ap[0], [1, bt * F]])
                                xr_o = AP(xr_g.tensor, xr_g.offset + j0 * F,
                                          [xr_g.ap[0], [1, bt * F]])
                            elif gb:
                                xl_o = xl_g[:, j0:j0 + bt, :]
                                xr_o = xr_g[:, j0:j0 + bt, :]
                            else:
                                xl_o = xl_g[:, j0, :]
                                xr_o = xr_g[:, j0, :]
                            nc.gpsimd.indirect_dma_start(
                                out=xl_o, out_offset=None,
                                in_=xl_full[li][:],
                                in_offset=IndirectOffsetOnAxis(
                                    ap=srcm_sb[:, tc0:tc0 + bt], axis=0))
                            nc.gpsimd.indirect_dma_start(
                                out=xr_o, out_offset=None,
                                in_=xr_own[li][:],
                                in_offset=IndirectOffsetOnAxis(
                                    ap=drowm_sb[:, tc0:tc0 + bt], axis=0))
                        # indicator IT[p, jj, n] = (iota[n] == dpos[p, col])
                        it_sb = gp.tile([P, NTH, P], dt.float16, tag="it" + tg,
                                        name="it")
                        iota_b = AP(iota_sb.tensor, iota_sb.offset,
                                    [iota_sb.ap[0], [0, nth], [1, P]])
                        dp = dposm_sb[:, bb * NT + g0:bb * NT + g0 + nth]
                        dpos_b = AP(dp.tensor, dp.offset, [dp.ap[0], [1, nth], [0, P]])
                        nc.vector.tensor_tensor(out=it_sb[:, :nth, :], in0=iota_b,
                                                in1=dpos_b,
                                                op=mybir.AluOpType.is_equal)
                        # z = xl + xr into zl_sb (no aliasing on DVE)
                        zl_sb = gp.tile([P, NTH, F], dt.float16, tag="zl" + tg,
                                        name="zl")
                        nc.vector.tensor_tensor(out=zl_sb[:, :nth, :],
                                                in0=xl_g[:, :nth, :],
                                                in1=xr_g[:, :nth, :],
                                                op=mybir.AluOpType.add)
                        # leaky relu via Prelu with alpha AP; xr_g is dead,
                        # reuse it for the Prelu output
                        nc.scalar.activation(xr_g[:, :nth, :], zl_sb[:, :nth, :],
                                             mybir.ActivationFunctionType.Prelu,
                                             alpha=alpha_sb[:])
                        # zw = zl * att (into xr_g scratch), logits = sum_c zw
                        ab = attb_sb[li]
                        attb_4d = AP(ab.tensor, ab.offset,
                                     [ab.ap[0], [0, nth], [chan, nh], [1, chan]])
                        zl_4d = AP(xr_g.tensor, xr_g.offset,
                                   [xr_g.ap[0], [F, nth], [chan, nh], [1, chan]])
                        zw_4d = AP(zl_sb.tensor, zl_sb.offset,
                                   [zl_sb.ap[0], [F, nth], [chan, nh], [1, chan]])
                        nc.vector.tensor_tensor(out=zw_4d, in0=zl_4d, in1=attb_4d,
                                                op=mybir.AluOpType.mult)
                        logit_sb = gp.tile([P, NTH, nh], f32, tag="logit" + tg, name="logit")
                        nc.vector.tensor_reduce(logit_sb[:, :nth, :], zw_4d,
                                                axis=mybir.AxisListType.X,
                                                op=mybir.AluOpType.add)
                        # rhs = [xl*w | w]
                        rhs_sb = gp.tile([P, NTH, FD], dt.float16, tag="rhs" + tg,
                                         name="rhs")
                        nc.scalar.activation(rhs_sb[:, :nth, F:FD],
                                             logit_sb[:, :nth, :],
                                             mybir.ActivationFunctionType.Exp)
                        w_b = AP(rhs_sb.tensor, rhs_sb.offset + F,
                                 [rhs_sb.ap[0], [FD, nth], [1, nh], [0, chan]])
                        xl_4d = AP(xl_g.tensor, xl_g.offset,
                                   [xl_g.ap[0], [F, nth], [chan, nh], [1, chan]])
                        rhs_4d = AP(rhs_sb.tensor, rhs_sb.offset,
                                    [rhs_sb.ap[0], [FD, nth], [chan, nh], [1, chan]])
                        nc.vector.tensor_tensor(out=rhs_4d, in0=xl_4d, in1=w_b,
                                                op=mybir.AluOpType.mult)
                        # segment matmul: [num | den] accumulated over NT tiles
                        for jj in range(nth):
                            j = g0 + jj
                            nc.tensor.matmul(num_ps[:],
                                             it_sb[:, jj, :],
                                             rhs_sb[:, jj, :],
                                             start=(j == 0), stop=(j == NT - 1))
                    # out = num / max(den, tiny) + bias
                    den_sb = gp.tile([P, nh], f32, tag="den", name="den")
                    nc.vector.tensor_scalar_max(den_sb[:], num_ps[:, F:FD], 1e-30)
                    rec_sb = gp.tile([P, nh], f32, tag="rec", name="rec")
                    nc.vector.reciprocal(rec_sb[:], den_sb[:])
                    ov_sb = gp.tile([P, F], f32, tag="ov", name="ov")
                    rec_b = AP(rec_sb.tensor, rec_sb.offset,
                               [rec_sb.ap[0], [1, nh], [0, chan]])
                    num_3d = AP(num_ps.tensor, num_ps.offset,
                                [num_ps.ap[0], [chan, nh], [1, chan]])
                    nc.vector.tensor_tensor(
                        out=AP(ov_sb.tensor, ov_sb.offset,
                               [ov_sb.ap[0], [chan, nh], [1, chan]]),
                        in0=num_3d, in1=rec_b, op=mybir.AluOpType.mult)
                    hv_sb = gp.tile([P, F], f32, tag="hv", name="hv")
                    nc.vector.tensor_tensor(out=hv_sb[:], in0=ov_sb[:],
                                            in1=bb_sb[li][:],
                                            op=mybir.AluOpType.add)
                    if li < 2:
                        # elu = relu(h) + exp(min(h,0)) - 1, then h^T to DRAM
                        mn_sb = gp.tile([P, F], f32, tag="mn", name="mn")
                        nc.vector.tensor_scalar_min(mn_sb[:], hv_sb[:], 0.0)
                        ex_sb = gp.tile([P, F], f32, tag="ex", name="ex")
                        nc.scalar.activation(ex_sb[:], mn_sb[:],
                                             mybir.ActivationFunctionType.Exp)
                        rl_sb = gp.tile([P, F], f32, tag="rl", name="rl")
                        nc.scalar.activation(rl_sb[:], hv_sb[:],
                                             mybir.ActivationFunctionType.Relu)
                        el_sb = gp.tile([P, F], f32, tag="el", name="el")
                        nc.vector.tensor_tensor(out=el_sb[:], in0=rl_sb[:],
                                                in1=ex_sb[:],
                                                op=mybir.AluOpType.add)
                        nc.vector.tensor_scalar_add(el_sb[:], el_sb[:], -1.0)
                        for half in range(2):
                            tp_ps = tps.tile([P, P], f32, space="PSUM", tag="tp", name="tp")
                            nc.tensor.transpose(
                                tp_ps[:], el_sb[:, half * P:(half + 1) * P],
                                ident_sb[:])
                            tp_sb = gp.tile([P, P], dt.float16, tag="tpsb", name="tpsb")
                            nc.vector.tensor_copy(tp_sb[:], tp_ps[:])
                            nc.sync.dma_start(
                                hT_dram[li][half * P:(half + 1) * P,
                                            bb * P:(bb + 1) * P], tp_sb[:])
                    else:
                        # log_softmax over 47 classes
                        mx_sb = gp.tile([P, 1], f32, tag="mx", name="mx")
                        nc.vector.tensor_reduce(mx_sb[:], hv_sb[:],
                                                axis=mybir.AxisListType.X,
                                                op=mybir.AluOpType.max,
                                                negate=True)
                        e2_sb = gp.tile([P, F], f32, tag="e2", name="e2")
                        sm_sb = gp.tile([P, 1], f32, tag="sm", name="sm")
                        nc.scalar.activation(e2_sb[:, :NCLASS], hv_sb[:],
                                             mybir.ActivationFunctionType.Exp,
                                             bias=mx_sb[:], accum_out=sm_sb[:])
                        ln_sb = gp.tile([P, 1], f32, tag="ln", name="ln")
                        nc.scalar.activation(ln_sb[:], sm_sb[:],
                                             mybir.ActivationFunctionType.Ln)
                        sh_sb = gp.tile([P, 1], f32, tag="sh", name="sh")
                        nc.vector.tensor_tensor(out=sh_sb[:], in0=mx_sb[:],
                                                in1=ln_sb[:],
                                                op=mybir.AluOpType.subtract)
                        fo_sb = gp.tile([P, F], dt.float16, tag="fo", name="fo")
                        nc.vector.tensor_scalar(fo_sb[:, :NCLASS], hv_sb[:],
                                                sh_sb[:], None,
                                                op0=mybir.AluOpType.add)
                        nc.sync.dma_start(out_own[bb * P:(bb + 1) * P, :],
                                          fo_sb[:, :NCLASS])

            edge_phase(0, HID, H8, C32)

            # ---- node phase layer 1 + AllGather ----
            def feed_hT(li):
                def f(t, k):
                    s = mp.tile([P, P], dt.float16, tag="hfeed", name="hfeed")
                    nc.sync.dma_start(
                        s[:], hT_dram[li][k * P:(k + 1) * P, t * P:(t + 1) * P])
                    return s[:]
                return f
            node_matmuls(feed_hT(0), 2, HID, w_sb[2], xl_bounce[1], BPC)
            allgather(1)
            node_matmuls(feed_hT(0), 2, HID, w_sb[3], xr_own[1], BPC)

            edge_phase(1, HID, H8, C32)

            node_matmuls(feed_hT(1), 2, NCLASS, w_sb[4], xl_bounce[2], BPC)
            allgather(2)
            node_matmuls(feed_hT(1), 2, NCLASS, w_sb[5], xr_own[2], BPC)

            edge_phase(2, NCLASS, 1, NCLASS)

            # ---- gather all per-core outputs + permute to node order ----
            # (host then fetches a single core's out_final shard: 1 RPC)
            out_slots = nc.dram_tensor("out_slots", [NSLOT, NCLASS],
                                       dt.float16, **shared).ap()
            if sim:
                nc.sync.dma_start(out_slots[:NCPAD, :], out_own[:])
            else:
                nc.gpsimd.collective_compute(
                    "AllGather", mybir.AluOpType.bypass,
                    ins=[out_own.opt()], outs=[out_slots.opt()],
                    replica_groups=[list(range(NCORES))])
            if gb:
                OCH = 6
                for t0 in range(0, NTO, OCH):
                    bt = min(OCH, NTO - t0)
                    og = gp.tile([P, OCH, NCLASS], dt.float16, tag="og", name="og")
                    nc.gpsimd.indirect_dma_start(
                        out=og[:, :bt, :], out_offset=None, in_=out_slots[:],
                        in_offset=IndirectOffsetOnAxis(
                            ap=permm_sb[:, t0:t0 + bt], axis=0))
                    ov = AP(out_final.tensor,
                            out_final.offset + t0 * P * NCLASS,
                            [[NCLASS, P], [P * NCLASS, bt], [1, NCLASS]])
                    nc.sync.dma_start(ov, og[:, :bt, :])
            else:
                for t in range(NTO):
                    og = gp.tile([P, NCLASS], dt.float16, tag="og", name="og")
                    nc.gpsimd.indirect_dma_start(
                        out=og[:], out_offset=None, in_=out_slots[:],
                        in_offset=IndirectOffsetOnAxis(
                            ap=permm_sb[:, t:t + 1], axis=0))
                    nc.sync.dma_start(out_final[t * P:(t + 1) * P, :], og[:])

    nc.compile()
    return nc


# ---------------------------------------------------------------------------
# Cached executor: build + jit once per NT; inputs stay device-resident.
# ---------------------------------------------------------------------------

_EXEC_CACHE = {}
_STATE = {"edge_copy": None, "layout": None, "perm": None,
          "in_copy": None, "dev_args": None, "dev_NT": None}


def _same(a, b):
    return a is not None and a.shape == b.shape and a.dtype == b.dtype \
        and np.array_equal(a, b)


def _get_exec(NT):
    if NT in _EXEC_CACHE:
        return _EXEC_CACHE[NT]
    bass2jax.install_neuronx_cc_hook()
    nc = _build(NT)

    partition_name = (nc.partition_id_tensor.name
                      if nc.partition_id_tensor else None)
    in_names, out_names, out_avals = [], [], []
    for alloc in nc.m.functions[0].allocations:
        if not isinstance(alloc, mybir.MemoryLocationSet):
            continue
        name = alloc.memorylocations[0].name
        if alloc.kind == "ExternalInput":
            if name != partition_name:
                in_names.append(name)
        elif alloc.kind == "ExternalOutput":
            out_names.append(name)
            out_avals.append(jax.core.ShapedArray(
                tuple(alloc.tensor_shape), mybir.dt.np(alloc.dtype)))
    assert nc.dbg_addr is None
    n_params, n_outs = len(in_names), len(out_names)
    all_in = tuple(in_names + out_names +
                   ([partition_name] if partition_name else []))

    def _body(*args):
        operands = list(args)
        if partition_name is not None:
            operands.append(bass2jax.partition_id_tensor())
        outs = bass2jax._bass_exec_p.bind(
            *operands, out_avals=tuple(out_avals), in_names=all_in,
            out_names=tuple(out_names), lowering_input_output_aliases=(),
            sim_require_finite=True, sim_require_nnan=True, nc=nc)
        return tuple(outs)

    devices = jax.devices()[:NCORES]
    mesh = Mesh(np.asarray(devices), ("core",))
    spec = PartitionSpec("core")
    sharding = NamedSharding(mesh, spec)
    sharded = jax.jit(
        shard_map(_body, mesh=mesh, in_specs=(spec,) * (n_params + n_outs),
                  out_specs=(spec,) * n_outs, check_rep=False),
        keep_unused=True)

    # The NEFF writes every element of out_own, so the "output" operands
    # need not be donated — keep one persistent zeros array device-resident
    # and pass it on every call.
    zinfo = [(tuple(a.shape), a.dtype) for a in out_avals]
    zeros_arr = tuple(
        jax.device_put(np.zeros((NCORES * s[0],) + s[1:], d), sharding)
        for s, d in zinfo)

    ex = dict(nc=nc, jit=sharded, zeros_arr=zeros_arr, in_names=in_names,
              out_names=out_names, out_avals=out_avals, sharding=sharding)
    _EXEC_CACHE[NT] = ex
    return ex


def _concat_inputs(ex, user, layout):
    """Build the global (8*rows, ...) arrays in in_names order."""
    NT, src_m, dpos_m, drow_m, core_of, bb_of, pos_of, gslot = layout
    TPC = BPC * NT
    xT = np.zeros((P, NSLOT), np.float32)
    xT[:, gslot] = user["x"].T
    xTown_g = np.ascontiguousarray(
        xT.reshape(P, NCORES, NCPAD).transpose(1, 0, 2)).reshape(
            NCORES * P, NCPAD)

    def rep(a):
        a = np.ascontiguousarray(np.asarray(a, np.float32))
        return np.ascontiguousarray(
            np.broadcast_to(a[None], (NCORES,) + a.shape)).reshape(
                (NCORES * a.shape[0],) + a.shape[1:])

    def bc(a, w):
        return np.broadcast_to(
            np.asarray(a, np.float32).reshape(1, w), (P, w))

    iota = np.broadcast_to(np.arange(P, dtype=np.float32)[None, :], (P, P))
    arrs = {
        "xTown": xTown_g,
        "wl0": rep(user["Wl0"]), "wr0": rep(user["Wr0"]),
        "wl1": rep(user["Wl1"]), "wr1": rep(user["Wr1"]),
        "wl2": rep(user["Wl2"]), "wr2": rep(user["Wr2"]),
        "attb0": rep(bc(user["a0"], HID)), "attb1": rep(bc(user["a1"], HID)),
        "attb2": rep(bc(user["a2"], NCLASS)),
        "bb0": rep(bc(user["b0"], HID)), "bb1": rep(bc(user["b1"], HID)),
        "bb2": rep(bc(user["b2"], NCLASS)),
        "iota": rep(iota), "ident": rep(np.eye(P, dtype=np.float32)),
        "srcm": src_m.reshape(NCORES * P, TPC),
        "dposm": dpos_m.reshape(NCORES * P, TPC),
        "drowm": drow_m.reshape(NCORES * P, TPC),
    }
    pflat = np.zeros(NTO * P, np.int32)
    pflat[:N] = (core_of * NCPAD + bb_of * P + pos_of).astype(np.int32)
    pm = np.ascontiguousarray(pflat.reshape(NTO, P).T)        # [P, NTO]
    arrs["permm"] = np.ascontiguousarray(
        np.broadcast_to(pm[None], (NCORES,) + pm.shape)).reshape(
            NCORES * P, NTO)
    host = [arrs[n] for n in ex["in_names"]]
    return jax.device_put(host, [ex["sharding"]] * len(host))


def kernel(x, edge_index, Wl0, Wr0, a0, b0, Wl1, Wr1, a1, b1, Wl2, Wr2, a2, b2):
    x = np.ascontiguousarray(np.asarray(x, np.float32))
    edge_index = np.ascontiguousarray(np.asarray(edge_index))
    user = dict(x=x, Wl0=Wl0, Wr0=Wr0, a0=a0, b0=b0, Wl1=Wl1, Wr1=Wr1,
                a1=a1, b1=b1, Wl2=Wl2, Wr2=Wr2, a2=a2, b2=b2)
    user = {k: np.ascontiguousarray(np.asarray(v, np.float32))
            for k, v in user.items()}
    st = _STATE

    # Optimistic dispatch: if we have cached device inputs from a previous
    # call, launch the NEFF immediately and validate the cache against the
    # new inputs while the device runs. On mismatch the speculative result
    # is discarded and we re-dispatch with fresh uploads.
    outs = None
    if st["dev_args"] is not None:
        ex = _get_exec(st["dev_NT"])
        outs = ex["jit"](*st["dev_args"], *ex["zeros_arr"])

    edge_ok = _same(st["edge_copy"], edge_index)
    if not edge_ok:
        st["layout"] = _layout(edge_index)
        st["edge_copy"] = edge_index.copy()
        st["dev_args"] = None

    NT = st["layout"][0]
    ex = _get_exec(NT)

    if (st["dev_args"] is None or st["dev_NT"] != NT or
            st["in_copy"] is None or
            not all(_same(st["in_copy"].get(k), user[k]) for k in user)):
        st["dev_args"] = _concat_inputs(ex, user, st["layout"])
        st["in_copy"] = {k: v.copy() for k, v in user.items()}
        st["dev_NT"] = NT
        outs = None  # speculative result was stale

    # out_final is replicated across cores (post-AllGather + on-device
    # permute); fetch a single core's shard — one transfer RPC. Retry once
    # on transient device/transport errors.
    last_err = None
    for attempt in range(2):
        try:
            if outs is None:
                outs = ex["jit"](*st["dev_args"], *ex["zeros_arr"])
            res = np.asarray(outs[0].addressable_shards[0].data)
            return res[:N].astype(np.float32)
        except Exception as e:                       # noqa: BLE001
            last_err = e
            outs = None
            import time
            time.sleep(2.0)
    raise last_err
